# revision 39
# baseline (speedup 1.0000x reference)
"""Multi-head attention (b=2, p=16, n=512, d=512, h=8, dh=64) on 8 TRN2 cores.

Data-parallel over the 32 (b,p) sequences: 4 sequences per core, no
collectives.  Per-core dataflow (everything "T" = feature-on-partition):

  xT  (d,n)  --W_qkv stationary-->  qT,kT (e,n)   [e-tile = 2 heads]
  xT chunks stationary, W_v moving ->  v natural (n,e) -> vaug (j,h,65)
  dots pair: head A on PE rows 0:64, head B on rows 64:128 -> the two
    K=64 matmuls run CONCURRENTLY (row-tiled 32x32 subarrays)
  exp: one ScalarE activation per jh step over the 4-bank dots tile
  oT[dh,i] (+ sums row 64) = vaug_h.T @ expT_h   (M=65, ones column)
  softmax denom: sums rows -> [32,512] sbuf -> DVE 32x32 stream
    transpose -> exact reciprocal on a [32,16,8] view -> transpose back
    -> R_t = P8_t.T @ rec (K=8 PE broadcast) -> oT *= R_t (DVE)
  yT = W_out.T @ oT + b  (f32 out, per-dt DMA to DRAM)

Schedule: ~10 dummy warmup matmuls keep the PE busy (and HAM at full
clock) while the first input DMAs land; QKV of sequence s+1 and the
output projection of sequence s-1 fill the PE between the dots/exp/
attnv dependency chains of sequence s.
"""

import os
import sys

import numpy as np

for _p in ("/opt/trn_rl_repo", "/root/.axon_site/_ro/trn_rl_repo"):
    if os.path.isdir(_p) and _p not in sys.path:
        sys.path.insert(0, _p)

import concourse.bass as bass  # noqa: E402
import concourse.mybir as mybir  # noqa: E402
from concourse import bacc  # noqa: E402
from concourse.tile import TileContext  # noqa: E402

F32 = mybir.dt.float32
BF16 = mybir.dt.bfloat16

N_CORES = 8
SEQ_PER_CORE = 4  # (b*p)=32 sequences / 8 cores
N = 512  # tokens per sequence
D = 512  # model dim
HEADS = 8
DH = 64
SCALE = DH**-0.5
NT = N // 128  # 4 token tiles
DT = D // 128  # 4 dim tiles

EXP_F = mybir.ActivationFunctionType.Exp
COPY_F = mybir.ActivationFunctionType.Copy
MULT = mybir.AluOpType.mult


def build_nc():
    """Build the per-core SPMD Bass program (same program on all 8 cores)."""
    nc = bacc.Bacc("TRN2", target_bir_lowering=False)

    xT = nc.declare_dram_parameter(
        "xT", [SEQ_PER_CORE, DT, 128, N], BF16, isOutput=False
    )
    wqkv = nc.declare_dram_parameter("wqkv", [DT, 128, 3 * D], BF16, isOutput=False)
    wout = nc.declare_dram_parameter("wout", [DT, 128, D], BF16, isOutput=False)
    bout = nc.declare_dram_parameter("bout", [D], F32, isOutput=False)
    p8d = nc.declare_dram_parameter("p8d", [8, 4, 128], BF16, isOutput=False)
    out = nc.declare_dram_parameter(
        "out", [SEQ_PER_CORE, DT, 128, N], F32, isOutput=True
    )

    with TileContext(nc) as tc:
        with (
            tc.tile_pool(name="consts", bufs=1) as cpool,
            tc.tile_pool(name="xin", bufs=2) as xpool,
            tc.tile_pool(name="qk", bufs=2) as qkpool,
            tc.tile_pool(name="vaug", bufs=2) as vpool,
            tc.tile_pool(name="expt", bufs=2) as epool,
            tc.tile_pool(name="ot", bufs=2) as opool,
            tc.tile_pool(name="small", bufs=1) as spool,
            tc.tile_pool(name="yout", bufs=3) as ypool,
            tc.tile_pool(name="psd", bufs=2, space="PSUM") as psd,
            tc.tile_pool(name="pso", bufs=1, space="PSUM") as pso,
            tc.tile_pool(name="psq", bufs=2, space="PSUM") as psq,
        ):
            # ---- input DMAs, finely chunked so compute starts early ----
            p8_sb = cpool.tile([8, 4, 128], BF16, tag="p8")
            nc.sync.dma_start(p8_sb[:], p8d[:])
            wq_sb = cpool.tile([128, DT, 3 * D], BF16, tag="wq")
            wq_r = wqkv.rearrange("t p e -> p t e")
            xts = {}
            xts[0] = xpool.tile([128, DT, N], BF16, tag="x", name="xt0")
            nc.sync.dma_start(xts[0][:, 0, :], xT[0, 0])
            nc.sync.dma_start(wq_sb[:, :, 0:128], wq_r[:, :, 0:128])
            for dt in range(1, DT):
                nc.sync.dma_start(xts[0][:, dt, :], xT[0, dt])
            for et in range(1, 8):
                nc.sync.dma_start(
                    wq_sb[:, :, et * 128 : (et + 1) * 128],
                    wq_r[:, :, et * 128 : (et + 1) * 128],
                )
            nc.sync.dma_start(wq_sb[:, :, 2 * D : 3 * D], wq_r[:, :, 2 * D : 3 * D])
            xts[1] = xpool.tile([128, DT, N], BF16, tag="x", name="xt1")
            for dt in range(DT):
                nc.sync.dma_start(xts[1][:, dt, :], xT[1, dt])
            wo_sb = cpool.tile([128, DT, D], BF16, tag="wo")
            nc.sync.dma_start(wo_sb[:], wout.rearrange("t p e -> p t e"))
            b_sb = cpool.tile([128, DT], F32, tag="b")
            nc.sync.dma_start(b_sb[:], bout.rearrange("(t p) -> p t", p=128))
            # ---- PE warmup: dummy matmuls while input DMAs land --------
            scratch = cpool.tile([128, 512], BF16, tag="warm")
            nc.vector.memset(scratch[:], 0.0)
            for _ in range(10):
                ps = psq.tile([128, 512], F32, tag="ps")
                nc.tensor.matmul(
                    ps[:],
                    lhsT=scratch[:, 0:128],
                    rhs=scratch[:],
                    start=True,
                    stop=True,
                )

            # ---- softmax-denominator scratch (bufs=1, memset once) -----
            sflat = spool.tile([1, 4, 2, N], F32, tag="sflat")
            batch128 = spool.tile([128, 32], F32, tag="batch")
            recB = spool.tile([128, 32], BF16, tag="recB")
            rec_sb = spool.tile([8, N], BF16, tag="rec")
            nc.vector.memset(rec_sb[:], 0.0)

            seq_tiles = {}
            oT_hist = {}
            pending_nb = []
            pending_tail = []

            def qkv_alloc(s):
                """Allocate per-seq tiles + start the xT DMA."""
                if s not in xts:
                    xt = xpool.tile([128, DT, N], BF16, tag="x", name=f"xt{s}")
                    for dt in range(DT):
                        nc.sync.dma_start(xt[:, dt, :], xT[s, dt])
                    xts[s] = xt
                q_sb = qkpool.tile([128, DT, N], BF16, tag="q")
                k_sb = qkpool.tile([128, DT, N], BF16, tag="k")
                vaug = vpool.tile([128, NT, HEADS, DH + 1], BF16, tag="v")
                nc.vector.memset(vaug[:, :, :, DH : DH + 1], 1.0)
                seq_tiles[s] = (xts[s], q_sb, k_sb, vaug)

            def qkv_etile(s, et):
                """One QKV output tile (et 0..3 q, 4..7 k, 8..11 = v
                n-tiles): 4 accumulating matmuls + evacuation."""
                xt, q_sb, k_sb, vaug = seq_tiles[s]
                ps = psq.tile([128, 512], F32, tag="ps")
                if et < 8:
                    for dt in range(DT):
                        nc.tensor.matmul(
                            ps[:],
                            lhsT=wq_sb[:, dt, et * 128 : (et + 1) * 128],
                            rhs=xt[:, dt, :],
                            start=(dt == 0),
                            stop=(dt == DT - 1),
                        )
                    if et < 4:
                        nc.vector.tensor_copy(q_sb[:, et, :], ps[:])
                    else:
                        nc.scalar.copy(k_sb[:, et - 4, :], ps[:])
                else:
                    nt = et - 8
                    for dt in range(DT):
                        nc.tensor.matmul(
                            ps[:],
                            lhsT=xt[:, dt, nt * 128 : (nt + 1) * 128],
                            rhs=wq_sb[:, dt, 2 * D : 3 * D],
                            start=(dt == 0),
                            stop=(dt == DT - 1),
                        )
                    nc.vector.tensor_copy(
                        vaug[:, nt, :, 0:DH],
                        ps.rearrange("p (h d) -> p h d", h=HEADS),
                    )

            def proj_dt(s, oT_, dt):
                """One output-projection d-tile: 4 matmuls + bias + DMA."""
                ps = psq.tile([128, 512], F32, tag="ps")
                for et in range(DT):
                    nc.tensor.matmul(
                        ps[:],
                        lhsT=wo_sb[:, et, dt * 128 : (dt + 1) * 128],
                        rhs=oT_[:, et, :],
                        start=(et == 0),
                        stop=(et == DT - 1),
                    )
                yt = ypool.tile([128, N], F32, tag="y")
                nc.vector.tensor_scalar_add(yt[:], ps[:], b_sb[:, dt : dt + 1])
                nc.sync.dma_start(out[s, dt], yt[:])

            def norm_front(s, t):
                """Reciprocal of the sums rows for head pair (2t, 2t+1):
                lane-spread via DMA reshape (no PE involvement)."""
                nc.sync.dma_start(
                    batch128[32 * t : 32 * t + 32, :], sflat[0:1, t, :, :]
                )
                with nc.allow_low_precision(reason="softmax recip bf16"):
                    nc.vector.reciprocal(
                        recB[32 * t : 32 * t + 32, :],
                        batch128[32 * t : 32 * t + 32, :],
                    )
                nc.sync.dma_start(
                    rec_sb[2 * t : 2 * t + 2, :], recB[32 * t : 32 * t + 32, :]
                )

            def norm_back(t, oT_):
                """PE-broadcast the reciprocals and multiply into oT.
                Emitted one pair later so the DMA chain never blocks PE."""
                Rp = psq.tile([128, 512], F32, tag="ps")
                nc.tensor.matmul(
                    Rp[:],
                    lhsT=p8_sb[:, t, :],
                    rhs=rec_sb[0:8, :],
                    start=True,
                    stop=True,
                )
                nc.vector.tensor_tensor(oT_[:, t, :], oT_[:, t, :], Rp[:], MULT)

            # ---- prologue: QKV for sequence 0 (pairs 0/1 + v only; the
            # rest rides in sequence 0's own filler slots) ---------------
            qkv_alloc(0)
            for et in (0, 4, 8, 9, 10, 11, 1, 5):
                qkv_etile(0, et)

            for s in range(SEQ_PER_CORE):
                _, q_sb, k_sb, vaug = seq_tiles[s]
                oT = opool.tile([128, DT, N], BF16, tag="o")
                if s + 1 < SEQ_PER_CORE:
                    qkv_alloc(s + 1)

                # filler units (4 matmuls each) interleaved into the pairs:
                # this sequence's own tail QKV e-tiles (pairs 2/3), QKV of
                # sequence s+1, and projection d-tiles of sequence s-1
                own = [
                    (lambda s_=s, e_=e: qkv_etile(s_, e_)) for e in (2, 6, 3, 7)
                ]
                nxt = []
                if s + 1 < SEQ_PER_CORE:
                    nxt = [
                        (lambda s_=s + 1, e_=e: qkv_etile(s_, e_))
                        for e in (0, 4, 8, 9, 10, 11, 1, 5)
                    ]
                prev = []
                if s > 0:
                    prev = [
                        (lambda s_=s - 1, o_=oT_hist[s - 1], d_=d: proj_dt(s_, o_, d_))
                        for d in range(DT)
                    ]
                if nxt:
                    fill = own[:2] + nxt[:1] + prev[:1] + own[2:] + nxt[1:3]
                    fill += prev[1:2] + nxt[3:5] + prev[2:3] + nxt[5:7]
                    fill += prev[3:4] + nxt[7:]
                else:
                    # no next-seq QKV: keep the proj units after the
                    # pending norm_back pop (they read normalized oT)
                    fill = own + prev
                fi = 0

                def filler(k):
                    nonlocal fi
                    for _ in range(k):
                        if fi < len(fill):
                            fill[fi]()
                            fi += 1

                for t in range(4):  # head pair (2t, 2t+1)
                    expP = epool.tile([128, 2, NT, N], BF16, tag="expP")
                    oAB = pso.tile([DH + 1, 2, N], F32, tag="oAB")

                    def dots(jh):
                        # jj-granular ping-pong: each j-tile's A/B pair gets
                        # its own 2-bank tile (bufs=2), so the WAR slack is
                        # two exp steps and each exp is only ~1.1us
                        for jj in range(2):
                            jt = 2 * jh + jj
                            dj = psd.tile([128, 2, 512], F32, tag="dAB")
                            nc.tensor.matmul(
                                dj[:, 0, :],
                                lhsT=k_sb[0:64, t, jt * 128 : (jt + 1) * 128],
                                rhs=q_sb[0:64, t, :],
                                start=True,
                                stop=True,
                            )
                            nc.tensor.matmul(
                                dj[:, 1, :],
                                lhsT=k_sb[64:128, t, jt * 128 : (jt + 1) * 128],
                                rhs=q_sb[64:128, t, :],
                                start=True,
                                stop=True,
                            )
                            nc.scalar.activation(
                                expP[:, :, jt, :],
                                dj[:, :, :],
                                EXP_F,
                                scale=SCALE,
                            )

                    def attnv(jh):
                        for jj in range(2):
                            jt = 2 * jh + jj
                            nc.tensor.matmul(
                                oAB[:, 0, :],
                                lhsT=vaug[:, jt, 2 * t, :],
                                rhs=expP[:, 0, jt, :],
                                start=(jt == 0),
                                stop=(jt == NT - 1),
                            )
                            nc.tensor.matmul(
                                oAB[:, 1, :],
                                lhsT=vaug[:, jt, 2 * t + 1, :],
                                rhs=expP[:, 1, jt, :],
                                start=(jt == 0),
                                stop=(jt == NT - 1),
                            )

                    dots(0)
                    filler(3)
                    dots(1)
                    attnv(0)
                    if pending_nb:
                        pending_nb.pop()()
                    filler(1)
                    attnv(1)

                    # evacuate unnormalized oT (f32 PSUM -> bf16 SBUF)
                    nc.vector.tensor_copy(oT[0:64, t, :], oAB[0:64, 0, :])
                    nc.vector.tensor_copy(oT[64:128, t, :], oAB[0:64, 1, :])
                    # stash the softmax sums rows (both heads in one copy)
                    nc.vector.tensor_copy(sflat[0:1, t, :, :], oAB[64:65, :, :])
                    norm_front(s, t)
                    pending_nb.append(lambda t_=t, o_=oT: norm_back(t_, o_))

                oT_hist[s] = oT

                if s == SEQ_PER_CORE - 1:
                    # last sequence: overlap the et<3 projection partials
                    # (into the freed dots banks) with the final norm chain;
                    # only the et=3 matmuls and bias/DMA wait on the last
                    # multiply.
                    projp1 = psd.tile([128, 2, 512], F32, tag="dAB")
                    projp2 = psd.tile([128, 2, 512], F32, tag="dAB")
                    projps = [projp1[:, 0, :], projp1[:, 1, :],
                              projp2[:, 0, :], projp2[:, 1, :]]
                    for dt in range(DT):
                        for et in range(DT - 1):
                            nc.tensor.matmul(
                                projps[dt],
                                lhsT=wo_sb[:, et, dt * 128 : (dt + 1) * 128],
                                rhs=oT[:, et, :],
                                start=(et == 0),
                                stop=False,
                            )
                    if pending_nb:
                        pending_nb.pop()()
                    for dt in range(DT):
                        nc.tensor.matmul(
                            projps[dt],
                            lhsT=wo_sb[:, DT - 1, dt * 128 : (dt + 1) * 128],
                            rhs=oT[:, DT - 1, :],
                            start=False,
                            stop=True,
                        )
                    for dt in range(DT):
                        yt = ypool.tile([128, N], F32, tag="y", name=f"yt3{dt}")
                        if dt % 2 == 0:
                            nc.vector.tensor_scalar_add(
                                yt[:], projps[dt], b_sb[:, dt : dt + 1]
                            )
                            nc.sync.dma_start(out[s, dt], yt[:])
                        else:
                            nc.scalar.activation(
                                yt[:],
                                projps[dt],
                                mybir.ActivationFunctionType.Identity,
                                bias=b_sb[:, dt : dt + 1],
                                scale=1.0,
                            )
                            nc.scalar.dma_start(out[s, dt], yt[:])
            # projection of sequence 2 ran as filler inside sequence 3

    nc.compile()
    return nc


def make_in_maps(x, W_qkv, W_out, b_out):
    """Shard + lay out full inputs into the 8 per-core input maps."""
    import ml_dtypes

    b, p, n, d = x.shape
    xs = np.ascontiguousarray(x, dtype=np.float32).reshape(b * p, n, d)
    wqkv = (
        np.ascontiguousarray(W_qkv, dtype=np.float32)
        .reshape(DT, 128, 3 * D)
        .astype(ml_dtypes.bfloat16)
    )
    wout = (
        np.ascontiguousarray(W_out, dtype=np.float32)
        .reshape(DT, 128, D)
        .astype(ml_dtypes.bfloat16)
    )
    bo = np.ascontiguousarray(b_out, dtype=np.float32)

    p8 = np.zeros((8, 4, 128), dtype=ml_dtypes.bfloat16)
    for t in range(4):
        p8[2 * t, t, 0:64] = 1.0
        p8[2 * t + 1, t, 64:128] = 1.0

    in_maps = []
    for c in range(N_CORES):
        seqs = xs[c * SEQ_PER_CORE : (c + 1) * SEQ_PER_CORE]  # (4, n, d)
        xTl = (
            np.ascontiguousarray(seqs.transpose(0, 2, 1))
            .reshape(SEQ_PER_CORE, DT, 128, N)
            .astype(ml_dtypes.bfloat16)
        )
        in_maps.append(
            {"xT": xTl, "wqkv": wqkv, "wout": wout, "bout": bo, "p8d": p8}
        )
    return in_maps


def assemble_output(results, b, p, n, d):
    """Gather per-core yT outputs back into the full (b,p,n,d) array."""
    y = np.empty((b * p, n, d), dtype=np.float32)
    for c in range(N_CORES):
        yT = np.asarray(results[c]["out"]).reshape(SEQ_PER_CORE, D, N)
        y[c * SEQ_PER_CORE : (c + 1) * SEQ_PER_CORE] = yT.transpose(0, 2, 1)
    return y.reshape(b, p, n, d)


_NC_CACHE = None


def _get_nc():
    global _NC_CACHE
    if _NC_CACHE is None:
        _NC_CACHE = build_nc()
    return _NC_CACHE


def run(inputs, trace=False, **spmd_kwargs):
    """Run on the 8 NeuronCores; returns (full_output, BassKernelResults)."""
    from concourse.bass_utils import run_bass_kernel_spmd

    x = np.asarray(inputs["x"])
    b, p, n, d = x.shape
    nc = _get_nc()
    in_maps = make_in_maps(x, inputs["W_qkv"], inputs["W_out"], inputs["b_out"])
    res = run_bass_kernel_spmd(
        nc, in_maps, core_ids=list(range(N_CORES)), trace=trace, **spmd_kwargs
    )
    return assemble_output(res.results, b, p, n, d), res


def kernel(x, W_qkv, W_out, b_out):
    out, _ = run({"x": x, "W_qkv": W_qkv, "W_out": W_out, "b_out": b_out})
    return out.astype(np.float32)


# revision 40
# speedup vs baseline: 1.0263x; 1.0263x over previous
"""Multi-head attention (b=2, p=16, n=512, d=512, h=8, dh=64) on 8 TRN2 cores.

Data-parallel over the 32 (b,p) sequences: 4 sequences per core, no
collectives.  Per-core dataflow (everything "T" = feature-on-partition):

  xT  (d,n)  --W_qkv stationary-->  qT,kT (e,n)   [e-tile = 2 heads]
  xT chunks stationary, W_v moving ->  v natural (n,e) -> vaug (j,h,65)
  dots pair: head A on PE rows 0:64, head B on rows 64:128 -> the two
    K=64 matmuls run CONCURRENTLY (row-tiled 32x32 subarrays)
  exp: one ScalarE activation per jh step over the 4-bank dots tile
  oT[dh,i] (+ sums row 64) = vaug_h.T @ expT_h   (M=65, ones column)
  softmax denom: sums rows -> [32,512] sbuf -> DVE 32x32 stream
    transpose -> exact reciprocal on a [32,16,8] view -> transpose back
    -> R_t = P8_t.T @ rec (K=8 PE broadcast) -> oT *= R_t (DVE)
  yT = W_out.T @ oT + b  (f32 out, per-dt DMA to DRAM)

Schedule: ~10 dummy warmup matmuls keep the PE busy (and HAM at full
clock) while the first input DMAs land; QKV of sequence s+1 and the
output projection of sequence s-1 fill the PE between the dots/exp/
attnv dependency chains of sequence s.
"""

import os
import sys

import numpy as np

for _p in ("/opt/trn_rl_repo", "/root/.axon_site/_ro/trn_rl_repo"):
    if os.path.isdir(_p) and _p not in sys.path:
        sys.path.insert(0, _p)

import concourse.bass as bass  # noqa: E402
import concourse.mybir as mybir  # noqa: E402
from concourse import bacc  # noqa: E402
from concourse.tile import TileContext  # noqa: E402

F32 = mybir.dt.float32
BF16 = mybir.dt.bfloat16

N_CORES = 8
SEQ_PER_CORE = 4  # (b*p)=32 sequences / 8 cores
N = 512  # tokens per sequence
D = 512  # model dim
HEADS = 8
DH = 64
SCALE = DH**-0.5
NT = N // 128  # 4 token tiles
DT = D // 128  # 4 dim tiles

EXP_F = mybir.ActivationFunctionType.Exp
COPY_F = mybir.ActivationFunctionType.Copy
MULT = mybir.AluOpType.mult


def build_nc():
    """Build the per-core SPMD Bass program (same program on all 8 cores)."""
    nc = bacc.Bacc("TRN2", target_bir_lowering=False)

    xT = nc.declare_dram_parameter(
        "xT", [SEQ_PER_CORE, DT, 128, N], BF16, isOutput=False
    )
    wqkv = nc.declare_dram_parameter("wqkv", [DT, 128, 3 * D], BF16, isOutput=False)
    wout = nc.declare_dram_parameter("wout", [DT, 128, D], BF16, isOutput=False)
    bout = nc.declare_dram_parameter("bout", [D], F32, isOutput=False)
    p8d = nc.declare_dram_parameter("p8d", [8, 4, 128], BF16, isOutput=False)
    out = nc.declare_dram_parameter(
        "out", [SEQ_PER_CORE, DT, 128, N], F32, isOutput=True
    )

    with TileContext(nc) as tc:
        with (
            tc.tile_pool(name="consts", bufs=1) as cpool,
            tc.tile_pool(name="xin", bufs=2) as xpool,
            tc.tile_pool(name="qk", bufs=2) as qkpool,
            tc.tile_pool(name="vaug", bufs=2) as vpool,
            tc.tile_pool(name="expt", bufs=2) as epool,
            tc.tile_pool(name="ot", bufs=2) as opool,
            tc.tile_pool(name="small", bufs=1) as spool,
            tc.tile_pool(name="yout", bufs=3) as ypool,
            tc.tile_pool(name="psd", bufs=2, space="PSUM") as psd,
            tc.tile_pool(name="pso", bufs=1, space="PSUM") as pso,
            tc.tile_pool(name="psq", bufs=2, space="PSUM") as psq,
        ):
            # ---- input DMAs, finely chunked so compute starts early ----
            p8_sb = cpool.tile([8, 4, 128], BF16, tag="p8")
            nc.sync.dma_start(p8_sb[:], p8d[:])
            wq_sb = cpool.tile([128, DT, 3 * D], BF16, tag="wq")
            wq_r = wqkv.rearrange("t p e -> p t e")
            xts = {}
            xts[0] = xpool.tile([128, DT, N], BF16, tag="x", name="xt0")
            nc.sync.dma_start(xts[0][:, 0, :], xT[0, 0])
            nc.sync.dma_start(wq_sb[:, :, 0:128], wq_r[:, :, 0:128])
            for dt in range(1, DT):
                nc.sync.dma_start(xts[0][:, dt, :], xT[0, dt])
            for et in range(1, 8):
                nc.sync.dma_start(
                    wq_sb[:, :, et * 128 : (et + 1) * 128],
                    wq_r[:, :, et * 128 : (et + 1) * 128],
                )
            nc.sync.dma_start(wq_sb[:, :, 2 * D : 3 * D], wq_r[:, :, 2 * D : 3 * D])
            xts[1] = xpool.tile([128, DT, N], BF16, tag="x", name="xt1")
            for dt in range(DT):
                nc.sync.dma_start(xts[1][:, dt, :], xT[1, dt])
            wo_sb = cpool.tile([128, DT, D], BF16, tag="wo")
            nc.sync.dma_start(wo_sb[:], wout.rearrange("t p e -> p t e"))
            b_sb = cpool.tile([128, DT], F32, tag="b")
            nc.sync.dma_start(b_sb[:], bout.rearrange("(t p) -> p t", p=128))
            # ---- PE warmup: dummy matmuls while input DMAs land --------
            scratch = cpool.tile([128, 512], BF16, tag="warm")
            nc.vector.memset(scratch[:], 0.0)
            for _ in range(10):
                ps = psq.tile([128, 512], F32, tag="ps")
                nc.tensor.matmul(
                    ps[:],
                    lhsT=scratch[:, 0:128],
                    rhs=scratch[:],
                    start=True,
                    stop=True,
                )

            # ---- softmax-denominator scratch (bufs=1, memset once) -----
            sflat = spool.tile([1, 4, 2, N], F32, tag="sflat")
            batch128 = spool.tile([128, 32], F32, tag="batch")
            recB = spool.tile([128, 32], BF16, tag="recB")
            rec_sb = spool.tile([8, N], BF16, tag="rec")
            nc.vector.memset(rec_sb[:], 0.0)

            seq_tiles = {}
            oT_hist = {}
            pending_nb = []
            pending_tail = []

            def qkv_alloc(s):
                """Allocate per-seq tiles + start the xT DMA."""
                if s not in xts:
                    xt = xpool.tile([128, DT, N], BF16, tag="x", name=f"xt{s}")
                    for dt in range(DT):
                        nc.sync.dma_start(xt[:, dt, :], xT[s, dt])
                    xts[s] = xt
                q_sb = qkpool.tile([128, DT, N], BF16, tag="q")
                k_sb = qkpool.tile([128, DT, N], BF16, tag="k")
                vaug = vpool.tile([128, NT, HEADS, DH + 1], BF16, tag="v")
                nc.vector.memset(vaug[:, :, :, DH : DH + 1], 1.0)
                seq_tiles[s] = (xts[s], q_sb, k_sb, vaug)

            def qkv_etile(s, et):
                """One QKV output tile (et 0..3 q, 4..7 k, 8..11 = v
                n-tiles): 4 accumulating matmuls + evacuation."""
                xt, q_sb, k_sb, vaug = seq_tiles[s]
                ps = psq.tile([128, 512], F32, tag="ps")
                if et < 8:
                    for dt in range(DT):
                        nc.tensor.matmul(
                            ps[:],
                            lhsT=wq_sb[:, dt, et * 128 : (et + 1) * 128],
                            rhs=xt[:, dt, :],
                            start=(dt == 0),
                            stop=(dt == DT - 1),
                        )
                    if et < 4:
                        nc.vector.tensor_copy(q_sb[:, et, :], ps[:])
                    else:
                        nc.scalar.copy(k_sb[:, et - 4, :], ps[:])
                else:
                    nt = et - 8
                    for dt in range(DT):
                        nc.tensor.matmul(
                            ps[:],
                            lhsT=xt[:, dt, nt * 128 : (nt + 1) * 128],
                            rhs=wq_sb[:, dt, 2 * D : 3 * D],
                            start=(dt == 0),
                            stop=(dt == DT - 1),
                        )
                    nc.vector.tensor_copy(
                        vaug[:, nt, :, 0:DH],
                        ps.rearrange("p (h d) -> p h d", h=HEADS),
                    )

            def proj_dt(s, oT_, dt):
                """One output-projection d-tile: 4 matmuls + bias + DMA."""
                ps = psq.tile([128, 512], F32, tag="ps")
                for et in range(DT):
                    nc.tensor.matmul(
                        ps[:],
                        lhsT=wo_sb[:, et, dt * 128 : (dt + 1) * 128],
                        rhs=oT_[:, et, :],
                        start=(et == 0),
                        stop=(et == DT - 1),
                    )
                yt = ypool.tile([128, N], F32, tag="y")
                nc.vector.tensor_scalar_add(yt[:], ps[:], b_sb[:, dt : dt + 1])
                nc.sync.dma_start(out[s, dt], yt[:])

            def norm_front(s, t):
                """Reciprocal of the sums rows for head pair (2t, 2t+1):
                lane-spread via DMA reshape (no PE involvement)."""
                nc.sync.dma_start(
                    batch128[32 * t : 32 * t + 32, :], sflat[0:1, t, :, :]
                )
                with nc.allow_low_precision(reason="softmax recip bf16"):
                    nc.vector.reciprocal(
                        recB[32 * t : 32 * t + 32, :],
                        batch128[32 * t : 32 * t + 32, :],
                    )
                nc.sync.dma_start(
                    rec_sb[2 * t : 2 * t + 2, :], recB[32 * t : 32 * t + 32, :]
                )

            def norm_back(t, oT_):
                """PE-broadcast the reciprocals and multiply into oT.
                Emitted one pair later so the DMA chain never blocks PE."""
                Rp = psq.tile([128, 512], F32, tag="ps")
                nc.tensor.matmul(
                    Rp[:],
                    lhsT=p8_sb[:, t, :],
                    rhs=rec_sb[0:8, :],
                    start=True,
                    stop=True,
                )
                nc.vector.tensor_tensor(oT_[:, t, :], oT_[:, t, :], Rp[:], MULT)

            # ---- prologue: QKV for sequence 0 (pairs 0/1 + v only; the
            # rest rides in sequence 0's own filler slots) ---------------
            qkv_alloc(0)
            for et in (0, 4, 8, 9, 10, 11, 1, 5):
                qkv_etile(0, et)

            for s in range(SEQ_PER_CORE):
                _, q_sb, k_sb, vaug = seq_tiles[s]
                oT = opool.tile([128, DT, N], BF16, tag="o")
                if s + 1 < SEQ_PER_CORE:
                    qkv_alloc(s + 1)

                # filler units (4 matmuls each) interleaved into the pairs:
                # this sequence's own tail QKV e-tiles (pairs 2/3), QKV of
                # sequence s+1, and projection d-tiles of sequence s-1
                own = [
                    (lambda s_=s, e_=e: qkv_etile(s_, e_)) for e in (2, 6, 3, 7)
                ]
                nxt = []
                if s + 1 < SEQ_PER_CORE:
                    nxt = [
                        (lambda s_=s + 1, e_=e: qkv_etile(s_, e_))
                        for e in (0, 4, 8, 9, 10, 11, 1, 5)
                    ]
                prev = []
                if s > 0:
                    prev = [
                        (lambda s_=s - 1, o_=oT_hist[s - 1], d_=d: proj_dt(s_, o_, d_))
                        for d in range(DT)
                    ]
                if nxt:
                    fill = own[:2] + nxt[:1] + prev[:1] + own[2:] + nxt[1:3]
                    fill += prev[1:2] + nxt[3:5] + prev[2:3] + nxt[5:7]
                    fill += prev[3:4] + nxt[7:]
                else:
                    # no next-seq QKV: keep the proj units after the
                    # pending norm_back pop (they read normalized oT)
                    fill = own + prev
                fi = 0

                def filler(k):
                    nonlocal fi
                    for _ in range(k):
                        if fi < len(fill):
                            fill[fi]()
                            fi += 1

                for t in range(4):  # head pair (2t, 2t+1)
                    expP = epool.tile([128, 2, NT, N], BF16, tag="expP")
                    oAB = pso.tile([DH + 1, 2, N], F32, tag="oAB")

                    def dots(jh):
                        # jj-granular ping-pong: each j-tile's A/B pair gets
                        # its own 2-bank tile (bufs=2), so the WAR slack is
                        # two exp steps and each exp is only ~1.1us
                        for jj in range(2):
                            jt = 2 * jh + jj
                            dj = psd.tile([128, 2, 512], F32, tag="dAB")
                            nc.tensor.matmul(
                                dj[:, 0, :],
                                lhsT=k_sb[0:64, t, jt * 128 : (jt + 1) * 128],
                                rhs=q_sb[0:64, t, :],
                                start=True,
                                stop=True,
                            )
                            nc.tensor.matmul(
                                dj[:, 1, :],
                                lhsT=k_sb[64:128, t, jt * 128 : (jt + 1) * 128],
                                rhs=q_sb[64:128, t, :],
                                start=True,
                                stop=True,
                            )
                            nc.scalar.activation(
                                expP[:, :, jt, :],
                                dj[:, :, :],
                                EXP_F,
                                scale=SCALE,
                            )

                    def attnv(jh):
                        for jj in range(2):
                            jt = 2 * jh + jj
                            nc.tensor.matmul(
                                oAB[:, 0, :],
                                lhsT=vaug[:, jt, 2 * t, :],
                                rhs=expP[:, 0, jt, :],
                                start=(jt == 0),
                                stop=(jt == NT - 1),
                            )
                            nc.tensor.matmul(
                                oAB[:, 1, :],
                                lhsT=vaug[:, jt, 2 * t + 1, :],
                                rhs=expP[:, 1, jt, :],
                                start=(jt == 0),
                                stop=(jt == NT - 1),
                            )

                    dots(0)
                    filler(3)
                    dots(1)
                    attnv(0)
                    if pending_nb:
                        pending_nb.pop()()
                    filler(1)
                    attnv(1)

                    # evacuate unnormalized oT (f32 PSUM -> bf16 SBUF)
                    nc.vector.tensor_copy(oT[0:64, t, :], oAB[0:64, 0, :])
                    nc.vector.tensor_copy(oT[64:128, t, :], oAB[0:64, 1, :])
                    # stash the softmax sums rows (both heads in one copy)
                    nc.vector.tensor_copy(sflat[0:1, t, :, :], oAB[64:65, :, :])
                    norm_front(s, t)
                    pending_nb.append(lambda t_=t, o_=oT: norm_back(t_, o_))

                oT_hist[s] = oT

                if s == SEQ_PER_CORE - 1:
                    # last sequence: overlap the et<3 projection partials
                    # (into the freed dots banks) with the final norm chain;
                    # only the et=3 matmuls and bias/DMA wait on the last
                    # multiply.
                    projp1 = psd.tile([128, 2, 512], F32, tag="dAB")
                    projp2 = psd.tile([128, 2, 512], F32, tag="dAB")
                    projps = [projp1[:, 0, :], projp1[:, 1, :],
                              projp2[:, 0, :], projp2[:, 1, :]]
                    for dt in range(DT):
                        for et in range(DT - 1):
                            nc.tensor.matmul(
                                projps[dt],
                                lhsT=wo_sb[:, et, dt * 128 : (dt + 1) * 128],
                                rhs=oT[:, et, :],
                                start=(et == 0),
                                stop=False,
                            )
                    if pending_nb:
                        pending_nb.pop()()
                    for dt in range(DT):
                        nc.tensor.matmul(
                            projps[dt],
                            lhsT=wo_sb[:, DT - 1, dt * 128 : (dt + 1) * 128],
                            rhs=oT[:, DT - 1, :],
                            start=False,
                            stop=True,
                        )
                    for dt in range(DT):
                        yt = ypool.tile([128, N], F32, tag="y", name=f"yt3{dt}")
                        nc.vector.tensor_scalar_add(
                            yt[:], projps[dt], b_sb[:, dt : dt + 1]
                        )
                        nc.sync.dma_start(out[s, dt], yt[:])
            # projection of sequence 2 ran as filler inside sequence 3

    nc.compile()
    return nc


def make_in_maps(x, W_qkv, W_out, b_out):
    """Shard + lay out full inputs into the 8 per-core input maps."""
    import ml_dtypes

    b, p, n, d = x.shape
    xs = np.ascontiguousarray(x, dtype=np.float32).reshape(b * p, n, d)
    wqkv = (
        np.ascontiguousarray(W_qkv, dtype=np.float32)
        .reshape(DT, 128, 3 * D)
        .astype(ml_dtypes.bfloat16)
    )
    wout = (
        np.ascontiguousarray(W_out, dtype=np.float32)
        .reshape(DT, 128, D)
        .astype(ml_dtypes.bfloat16)
    )
    bo = np.ascontiguousarray(b_out, dtype=np.float32)

    p8 = np.zeros((8, 4, 128), dtype=ml_dtypes.bfloat16)
    for t in range(4):
        p8[2 * t, t, 0:64] = 1.0
        p8[2 * t + 1, t, 64:128] = 1.0

    in_maps = []
    for c in range(N_CORES):
        seqs = xs[c * SEQ_PER_CORE : (c + 1) * SEQ_PER_CORE]  # (4, n, d)
        xTl = (
            np.ascontiguousarray(seqs.transpose(0, 2, 1))
            .reshape(SEQ_PER_CORE, DT, 128, N)
            .astype(ml_dtypes.bfloat16)
        )
        in_maps.append(
            {"xT": xTl, "wqkv": wqkv, "wout": wout, "bout": bo, "p8d": p8}
        )
    return in_maps


def assemble_output(results, b, p, n, d):
    """Gather per-core yT outputs back into the full (b,p,n,d) array."""
    y = np.empty((b * p, n, d), dtype=np.float32)
    for c in range(N_CORES):
        yT = np.asarray(results[c]["out"]).reshape(SEQ_PER_CORE, D, N)
        y[c * SEQ_PER_CORE : (c + 1) * SEQ_PER_CORE] = yT.transpose(0, 2, 1)
    return y.reshape(b, p, n, d)


_NC_CACHE = None


def _get_nc():
    global _NC_CACHE
    if _NC_CACHE is None:
        _NC_CACHE = build_nc()
    return _NC_CACHE


def run(inputs, trace=False, **spmd_kwargs):
    """Run on the 8 NeuronCores; returns (full_output, BassKernelResults)."""
    from concourse.bass_utils import run_bass_kernel_spmd

    x = np.asarray(inputs["x"])
    b, p, n, d = x.shape
    nc = _get_nc()
    in_maps = make_in_maps(x, inputs["W_qkv"], inputs["W_out"], inputs["b_out"])
    res = run_bass_kernel_spmd(
        nc, in_maps, core_ids=list(range(N_CORES)), trace=trace, **spmd_kwargs
    )
    return assemble_output(res.results, b, p, n, d), res


def kernel(x, W_qkv, W_out, b_out):
    out, _ = run({"x": x, "W_qkv": W_qkv, "W_out": W_out, "b_out": b_out})
    return out.astype(np.float32)


# revision 41
# speedup vs baseline: 1.0312x; 1.0048x over previous
"""Multi-head attention (b=2, p=16, n=512, d=512, h=8, dh=64) on 8 TRN2 cores.

Data-parallel over the 32 (b,p) sequences: 4 sequences per core, no
collectives.  Per-core dataflow (everything "T" = feature-on-partition):

  xT  (d,n)  --W_qkv stationary-->  qT,kT (e,n)   [e-tile = 2 heads]
  xT chunks stationary, W_v moving ->  v natural (n,e) -> vaug (j,h,65)
  dots pair: head A on PE rows 0:64, head B on rows 64:128 -> the two
    K=64 matmuls run CONCURRENTLY (row-tiled 32x32 subarrays)
  exp: one ScalarE activation per jh step over the 4-bank dots tile
  oT[dh,i] (+ sums row 64) = vaug_h.T @ expT_h   (M=65, ones column)
  softmax denom: sums rows -> [32,512] sbuf -> DVE 32x32 stream
    transpose -> exact reciprocal on a [32,16,8] view -> transpose back
    -> R_t = P8_t.T @ rec (K=8 PE broadcast) -> oT *= R_t (DVE)
  yT = W_out.T @ oT + b  (f32 out, per-dt DMA to DRAM)

Schedule: ~10 dummy warmup matmuls keep the PE busy (and HAM at full
clock) while the first input DMAs land; QKV of sequence s+1 and the
output projection of sequence s-1 fill the PE between the dots/exp/
attnv dependency chains of sequence s.
"""

import os
import sys

import numpy as np

for _p in ("/opt/trn_rl_repo", "/root/.axon_site/_ro/trn_rl_repo"):
    if os.path.isdir(_p) and _p not in sys.path:
        sys.path.insert(0, _p)

import concourse.bass as bass  # noqa: E402
import concourse.mybir as mybir  # noqa: E402
from concourse import bacc  # noqa: E402
from concourse.tile import TileContext  # noqa: E402

F32 = mybir.dt.float32
BF16 = mybir.dt.bfloat16

N_CORES = 8
SEQ_PER_CORE = 4  # (b*p)=32 sequences / 8 cores
N = 512  # tokens per sequence
D = 512  # model dim
HEADS = 8
DH = 64
SCALE = DH**-0.5
NT = N // 128  # 4 token tiles
DT = D // 128  # 4 dim tiles

EXP_F = mybir.ActivationFunctionType.Exp
COPY_F = mybir.ActivationFunctionType.Copy
MULT = mybir.AluOpType.mult


def build_nc():
    """Build the per-core SPMD Bass program (same program on all 8 cores)."""
    nc = bacc.Bacc("TRN2", target_bir_lowering=False)

    xT = nc.declare_dram_parameter(
        "xT", [SEQ_PER_CORE, DT, 128, N], BF16, isOutput=False
    )
    wqkv = nc.declare_dram_parameter("wqkv", [DT, 128, 3 * D], BF16, isOutput=False)
    wout = nc.declare_dram_parameter("wout", [DT, 128, D], BF16, isOutput=False)
    bout = nc.declare_dram_parameter("bout", [D], F32, isOutput=False)
    p8d = nc.declare_dram_parameter("p8d", [8, 4, 128], BF16, isOutput=False)
    out = nc.declare_dram_parameter(
        "out", [SEQ_PER_CORE, DT, 128, N], F32, isOutput=True
    )

    with TileContext(nc) as tc:
        with (
            tc.tile_pool(name="consts", bufs=1) as cpool,
            tc.tile_pool(name="xin", bufs=2) as xpool,
            tc.tile_pool(name="qk", bufs=2) as qkpool,
            tc.tile_pool(name="vaug", bufs=2) as vpool,
            tc.tile_pool(name="expt", bufs=2) as epool,
            tc.tile_pool(name="ot", bufs=2) as opool,
            tc.tile_pool(name="small", bufs=1) as spool,
            tc.tile_pool(name="yout", bufs=3) as ypool,
            tc.tile_pool(name="psd", bufs=2, space="PSUM") as psd,
            tc.tile_pool(name="pso", bufs=1, space="PSUM") as pso,
            tc.tile_pool(name="psq", bufs=2, space="PSUM") as psq,
        ):
            # ---- input DMAs, finely chunked so compute starts early ----
            p8_sb = cpool.tile([8, 4, 128], BF16, tag="p8")
            nc.sync.dma_start(p8_sb[:], p8d[:])
            wq_sb = cpool.tile([128, DT, 3 * D], BF16, tag="wq")
            wq_r = wqkv.rearrange("t p e -> p t e")
            xts = {}
            xts[0] = xpool.tile([128, DT, N], BF16, tag="x", name="xt0")
            nc.sync.dma_start(xts[0][:, 0, :], xT[0, 0])
            nc.sync.dma_start(wq_sb[:, :, 0:128], wq_r[:, :, 0:128])
            for dt in range(1, DT):
                nc.sync.dma_start(xts[0][:, dt, :], xT[0, dt])
            for et in range(1, 8):
                nc.sync.dma_start(
                    wq_sb[:, :, et * 128 : (et + 1) * 128],
                    wq_r[:, :, et * 128 : (et + 1) * 128],
                )
            nc.sync.dma_start(wq_sb[:, :, 2 * D : 3 * D], wq_r[:, :, 2 * D : 3 * D])
            wo_sb = cpool.tile([128, DT, D], BF16, tag="wo")
            nc.sync.dma_start(wo_sb[:], wout.rearrange("t p e -> p t e"))
            b_sb = cpool.tile([128, DT], F32, tag="b")
            nc.sync.dma_start(b_sb[:], bout.rearrange("(t p) -> p t", p=128))
            # ---- PE warmup: dummy matmuls while input DMAs land --------
            scratch = cpool.tile([128, 512], BF16, tag="warm")
            nc.vector.memset(scratch[:], 0.0)
            for _ in range(10):
                ps = psq.tile([128, 512], F32, tag="ps")
                nc.tensor.matmul(
                    ps[:],
                    lhsT=scratch[:, 0:128],
                    rhs=scratch[:],
                    start=True,
                    stop=True,
                )

            # ---- softmax-denominator scratch (bufs=1, memset once) -----
            sflat = spool.tile([1, 4, 2, N], F32, tag="sflat")
            batch128 = spool.tile([128, 32], F32, tag="batch")
            recB = spool.tile([128, 32], BF16, tag="recB")
            rec_sb = spool.tile([8, N], BF16, tag="rec")
            nc.vector.memset(rec_sb[:], 0.0)

            seq_tiles = {}
            oT_hist = {}
            pending_nb = []
            pending_tail = []

            def qkv_alloc(s):
                """Allocate per-seq tiles + start the xT DMA."""
                if s not in xts:
                    xt = xpool.tile([128, DT, N], BF16, tag="x", name=f"xt{s}")
                    for dt in range(DT):
                        nc.sync.dma_start(xt[:, dt, :], xT[s, dt])
                    xts[s] = xt
                q_sb = qkpool.tile([128, DT, N], BF16, tag="q")
                k_sb = qkpool.tile([128, DT, N], BF16, tag="k")
                vaug = vpool.tile([128, NT, HEADS, DH + 1], BF16, tag="v")
                nc.vector.memset(vaug[:, :, :, DH : DH + 1], 1.0)
                seq_tiles[s] = (xts[s], q_sb, k_sb, vaug)

            def qkv_etile(s, et):
                """One QKV output tile (et 0..3 q, 4..7 k, 8..11 = v
                n-tiles): 4 accumulating matmuls + evacuation."""
                xt, q_sb, k_sb, vaug = seq_tiles[s]
                ps = psq.tile([128, 512], F32, tag="ps")
                if et < 8:
                    for dt in range(DT):
                        nc.tensor.matmul(
                            ps[:],
                            lhsT=wq_sb[:, dt, et * 128 : (et + 1) * 128],
                            rhs=xt[:, dt, :],
                            start=(dt == 0),
                            stop=(dt == DT - 1),
                        )
                    if et < 4:
                        nc.vector.tensor_copy(q_sb[:, et, :], ps[:])
                    else:
                        nc.scalar.copy(k_sb[:, et - 4, :], ps[:])
                else:
                    nt = et - 8
                    for dt in range(DT):
                        nc.tensor.matmul(
                            ps[:],
                            lhsT=xt[:, dt, nt * 128 : (nt + 1) * 128],
                            rhs=wq_sb[:, dt, 2 * D : 3 * D],
                            start=(dt == 0),
                            stop=(dt == DT - 1),
                        )
                    nc.vector.tensor_copy(
                        vaug[:, nt, :, 0:DH],
                        ps.rearrange("p (h d) -> p h d", h=HEADS),
                    )

            def proj_dt(s, oT_, dt):
                """One output-projection d-tile: 4 matmuls + bias + DMA."""
                ps = psq.tile([128, 512], F32, tag="ps")
                for et in range(DT):
                    nc.tensor.matmul(
                        ps[:],
                        lhsT=wo_sb[:, et, dt * 128 : (dt + 1) * 128],
                        rhs=oT_[:, et, :],
                        start=(et == 0),
                        stop=(et == DT - 1),
                    )
                yt = ypool.tile([128, N], F32, tag="y")
                nc.vector.tensor_scalar_add(yt[:], ps[:], b_sb[:, dt : dt + 1])
                nc.sync.dma_start(out[s, dt], yt[:])

            def norm_front(s, t):
                """Reciprocal of the sums rows for head pair (2t, 2t+1):
                lane-spread via DMA reshape (no PE involvement)."""
                nc.sync.dma_start(
                    batch128[32 * t : 32 * t + 32, :], sflat[0:1, t, :, :]
                )
                with nc.allow_low_precision(reason="softmax recip bf16"):
                    nc.vector.reciprocal(
                        recB[32 * t : 32 * t + 32, :],
                        batch128[32 * t : 32 * t + 32, :],
                    )
                nc.sync.dma_start(
                    rec_sb[2 * t : 2 * t + 2, :], recB[32 * t : 32 * t + 32, :]
                )

            def norm_back(t, oT_):
                """PE-broadcast the reciprocals and multiply into oT.
                Emitted one pair later so the DMA chain never blocks PE."""
                Rp = psq.tile([128, 512], F32, tag="ps")
                nc.tensor.matmul(
                    Rp[:],
                    lhsT=p8_sb[:, t, :],
                    rhs=rec_sb[0:8, :],
                    start=True,
                    stop=True,
                )
                nc.vector.tensor_tensor(oT_[:, t, :], oT_[:, t, :], Rp[:], MULT)

            # ---- prologue: QKV for sequence 0 (pairs 0/1 + v only; the
            # rest rides in sequence 0's own filler slots) ---------------
            qkv_alloc(0)
            for et in (0, 4, 8, 9, 10, 11, 1, 5):
                qkv_etile(0, et)

            for s in range(SEQ_PER_CORE):
                _, q_sb, k_sb, vaug = seq_tiles[s]
                oT = opool.tile([128, DT, N], BF16, tag="o")
                if s + 1 < SEQ_PER_CORE:
                    qkv_alloc(s + 1)

                # filler units (4 matmuls each) interleaved into the pairs:
                # this sequence's own tail QKV e-tiles (pairs 2/3), QKV of
                # sequence s+1, and projection d-tiles of sequence s-1
                own = [
                    (lambda s_=s, e_=e: qkv_etile(s_, e_)) for e in (2, 6, 3, 7)
                ]
                nxt = []
                if s + 1 < SEQ_PER_CORE:
                    nxt = [
                        (lambda s_=s + 1, e_=e: qkv_etile(s_, e_))
                        for e in (0, 4, 8, 9, 10, 11, 1, 5)
                    ]
                prev = []
                if s > 0:
                    prev = [
                        (lambda s_=s - 1, o_=oT_hist[s - 1], d_=d: proj_dt(s_, o_, d_))
                        for d in range(DT)
                    ]
                if nxt:
                    fill = own[:2] + nxt[:1] + prev[:1] + own[2:] + nxt[1:3]
                    fill += prev[1:2] + nxt[3:5] + prev[2:3] + nxt[5:7]
                    fill += prev[3:4] + nxt[7:]
                else:
                    # no next-seq QKV: keep the proj units after the
                    # pending norm_back pop (they read normalized oT)
                    fill = own + prev
                fi = 0

                def filler(k):
                    nonlocal fi
                    for _ in range(k):
                        if fi < len(fill):
                            fill[fi]()
                            fi += 1

                for t in range(4):  # head pair (2t, 2t+1)
                    expP = epool.tile([128, 2, NT, N], BF16, tag="expP")
                    oAB = pso.tile([DH + 1, 2, N], F32, tag="oAB")

                    def dots(jh):
                        # jj-granular ping-pong: each j-tile's A/B pair gets
                        # its own 2-bank tile (bufs=2), so the WAR slack is
                        # two exp steps and each exp is only ~1.1us
                        for jj in range(2):
                            jt = 2 * jh + jj
                            dj = psd.tile([128, 2, 512], F32, tag="dAB")
                            nc.tensor.matmul(
                                dj[:, 0, :],
                                lhsT=k_sb[0:64, t, jt * 128 : (jt + 1) * 128],
                                rhs=q_sb[0:64, t, :],
                                start=True,
                                stop=True,
                            )
                            nc.tensor.matmul(
                                dj[:, 1, :],
                                lhsT=k_sb[64:128, t, jt * 128 : (jt + 1) * 128],
                                rhs=q_sb[64:128, t, :],
                                start=True,
                                stop=True,
                            )
                            nc.scalar.activation(
                                expP[:, :, jt, :],
                                dj[:, :, :],
                                EXP_F,
                                scale=SCALE,
                            )

                    def attnv(jh):
                        for jj in range(2):
                            jt = 2 * jh + jj
                            nc.tensor.matmul(
                                oAB[:, 0, :],
                                lhsT=vaug[:, jt, 2 * t, :],
                                rhs=expP[:, 0, jt, :],
                                start=(jt == 0),
                                stop=(jt == NT - 1),
                            )
                            nc.tensor.matmul(
                                oAB[:, 1, :],
                                lhsT=vaug[:, jt, 2 * t + 1, :],
                                rhs=expP[:, 1, jt, :],
                                start=(jt == 0),
                                stop=(jt == NT - 1),
                            )

                    dots(0)
                    filler(3)
                    dots(1)
                    attnv(0)
                    if pending_nb:
                        pending_nb.pop()()
                    filler(1)
                    attnv(1)

                    # evacuate unnormalized oT (f32 PSUM -> bf16 SBUF)
                    nc.vector.tensor_copy(oT[0:64, t, :], oAB[0:64, 0, :])
                    nc.vector.tensor_copy(oT[64:128, t, :], oAB[0:64, 1, :])
                    # stash the softmax sums rows (both heads in one copy)
                    nc.vector.tensor_copy(sflat[0:1, t, :, :], oAB[64:65, :, :])
                    norm_front(s, t)
                    pending_nb.append(lambda t_=t, o_=oT: norm_back(t_, o_))

                oT_hist[s] = oT

                if s == SEQ_PER_CORE - 1:
                    # last sequence: overlap the et<3 projection partials
                    # (into the freed dots banks) with the final norm chain;
                    # only the et=3 matmuls and bias/DMA wait on the last
                    # multiply.
                    projp1 = psd.tile([128, 2, 512], F32, tag="dAB")
                    projp2 = psd.tile([128, 2, 512], F32, tag="dAB")
                    projps = [projp1[:, 0, :], projp1[:, 1, :],
                              projp2[:, 0, :], projp2[:, 1, :]]
                    for dt in range(DT):
                        for et in range(DT - 1):
                            nc.tensor.matmul(
                                projps[dt],
                                lhsT=wo_sb[:, et, dt * 128 : (dt + 1) * 128],
                                rhs=oT[:, et, :],
                                start=(et == 0),
                                stop=False,
                            )
                    if pending_nb:
                        pending_nb.pop()()
                    for dt in range(DT):
                        nc.tensor.matmul(
                            projps[dt],
                            lhsT=wo_sb[:, DT - 1, dt * 128 : (dt + 1) * 128],
                            rhs=oT[:, DT - 1, :],
                            start=False,
                            stop=True,
                        )
                    for dt in range(DT):
                        yt = ypool.tile([128, N], F32, tag="y", name=f"yt3{dt}")
                        nc.vector.tensor_scalar_add(
                            yt[:], projps[dt], b_sb[:, dt : dt + 1]
                        )
                        nc.sync.dma_start(out[s, dt], yt[:])
            # projection of sequence 2 ran as filler inside sequence 3

    nc.compile()
    return nc


def make_in_maps(x, W_qkv, W_out, b_out):
    """Shard + lay out full inputs into the 8 per-core input maps."""
    import ml_dtypes

    b, p, n, d = x.shape
    xs = np.ascontiguousarray(x, dtype=np.float32).reshape(b * p, n, d)
    wqkv = (
        np.ascontiguousarray(W_qkv, dtype=np.float32)
        .reshape(DT, 128, 3 * D)
        .astype(ml_dtypes.bfloat16)
    )
    wout = (
        np.ascontiguousarray(W_out, dtype=np.float32)
        .reshape(DT, 128, D)
        .astype(ml_dtypes.bfloat16)
    )
    bo = np.ascontiguousarray(b_out, dtype=np.float32)

    p8 = np.zeros((8, 4, 128), dtype=ml_dtypes.bfloat16)
    for t in range(4):
        p8[2 * t, t, 0:64] = 1.0
        p8[2 * t + 1, t, 64:128] = 1.0

    in_maps = []
    for c in range(N_CORES):
        seqs = xs[c * SEQ_PER_CORE : (c + 1) * SEQ_PER_CORE]  # (4, n, d)
        xTl = (
            np.ascontiguousarray(seqs.transpose(0, 2, 1))
            .reshape(SEQ_PER_CORE, DT, 128, N)
            .astype(ml_dtypes.bfloat16)
        )
        in_maps.append(
            {"xT": xTl, "wqkv": wqkv, "wout": wout, "bout": bo, "p8d": p8}
        )
    return in_maps


def assemble_output(results, b, p, n, d):
    """Gather per-core yT outputs back into the full (b,p,n,d) array."""
    y = np.empty((b * p, n, d), dtype=np.float32)
    for c in range(N_CORES):
        yT = np.asarray(results[c]["out"]).reshape(SEQ_PER_CORE, D, N)
        y[c * SEQ_PER_CORE : (c + 1) * SEQ_PER_CORE] = yT.transpose(0, 2, 1)
    return y.reshape(b, p, n, d)


_NC_CACHE = None


def _get_nc():
    global _NC_CACHE
    if _NC_CACHE is None:
        _NC_CACHE = build_nc()
    return _NC_CACHE


def run(inputs, trace=False, **spmd_kwargs):
    """Run on the 8 NeuronCores; returns (full_output, BassKernelResults)."""
    from concourse.bass_utils import run_bass_kernel_spmd

    x = np.asarray(inputs["x"])
    b, p, n, d = x.shape
    nc = _get_nc()
    in_maps = make_in_maps(x, inputs["W_qkv"], inputs["W_out"], inputs["b_out"])
    res = run_bass_kernel_spmd(
        nc, in_maps, core_ids=list(range(N_CORES)), trace=trace, **spmd_kwargs
    )
    return assemble_output(res.results, b, p, n, d), res


def kernel(x, W_qkv, W_out, b_out):
    out, _ = run({"x": x, "W_qkv": W_qkv, "W_out": W_out, "b_out": b_out})
    return out.astype(np.float32)
